# revision 7
# baseline (speedup 1.0000x reference)
"""Trainium2 Bass kernel for nn_CrossAttentionLayer (B=4, C=256, H=W=64).

Sharding: 8 cores = batch(4) x query-half(2). Each core computes a
[C, N/2] = [256, 2048] output shard from x1-half [256, 2048] and full
x2 [256, 4096] for its batch. BN / conv-bias / 1/sqrt(d) are folded into
the weights host-side. All big matmuls run in float32r (TF32-like full
rate on the PE); softmax skips the max-subtraction (energies are small,
fp32 exp is exact enough) and its normalization is folded to the end:

  out[c, m] = (sum_n v[c, n] * exp(S[m, n])) / (sum_n exp(S[m, n])) + bv[c]
"""

import numpy as np
from contextlib import ExitStack, nullcontext

import concourse.bass as bass
import concourse.bacc as bacc
import concourse.mybir as mybir
import concourse.tile as tile
from concourse.bass_utils import run_bass_kernel_spmd

dt = mybir.dt
F32, F32R = dt.float32, dt.float32r
EPS = 1e-5
B, C, Hs, Ws = 4, 256, 64, 64
N = Hs * Ws            # 4096 spatial positions
DQK = C // 8           # 32
NCORES = 8
MH = N // 2            # 2048 query rows per core
NT = N // 128          # 32 key-side n-tiles
MQS = 512              # m-chunk (one PSUM bank of fp32)
MQ = MH // MQS         # 4 m-chunks

_prog = None
LAST_RESULTS = None


def _build(reps=1):
    nc = bacc.Bacc("TRN2", target_bir_lowering=False, debug=False)
    x1s = nc.dram_tensor("x1s", [C, MH], F32, kind="ExternalInput")
    x2s = nc.dram_tensor("x2s", [C, N], F32, kind="ExternalInput")
    wqT = nc.dram_tensor("wqT", [C, DQK], F32, kind="ExternalInput")
    wkT = nc.dram_tensor("wkT", [C, DQK], F32, kind="ExternalInput")
    wvT = nc.dram_tensor("wvT", [C, C], F32, kind="ExternalInput")
    bqd = nc.dram_tensor("bq", [DQK, 1], F32, kind="ExternalInput")
    bkd = nc.dram_tensor("bk", [DQK, 1], F32, kind="ExternalInput")
    bvd = nc.dram_tensor("bv", [C, 1], F32, kind="ExternalInput")
    y = nc.dram_tensor("y", [C, MH], F32, kind="ExternalOutput")

    Exp = mybir.ActivationFunctionType.Exp

    with tile.TileContext(nc) as tc, ExitStack() as ctx:
        sbc = ctx.enter_context(tc.tile_pool(name="sbc", bufs=1))
        sbx = ctx.enter_context(tc.tile_pool(name="sbx", bufs=2))
        sbp = ctx.enter_context(tc.tile_pool(name="sbp", bufs=3))
        psa = ctx.enter_context(tc.tile_pool(name="psa", bufs=3, space="PSUM"))
        pso = ctx.enter_context(tc.tile_pool(name="pso", bufs=4, space="PSUM"))
        psr = ctx.enter_context(tc.tile_pool(name="psr", bufs=1, space="PSUM"))
        if reps > 1:  # benchmarking: repeat the whole body on-device
            ctx.enter_context(tc.For_i(
                0, reps, 1,
                hint_engines=(mybir.EngineType.PE, mybir.EngineType.Activation,
                              mybir.EngineType.DVE, mybir.EngineType.SP),
            ))

        # constants / biases
        ones_f = sbc.tile([128, 1], F32, tag="ones_f")
        nc.vector.memset(ones_f[:], 1.0)
        ones_r = sbc.tile([128, 1], F32R, tag="ones_r")
        nc.vector.tensor_copy(ones_r[:], ones_f[:])
        bq_t = sbc.tile([DQK, 1], F32, tag="bq")
        nc.sync.dma_start(bq_t[:], bqd.ap())
        bk_t = sbc.tile([DQK, 1], F32, tag="bk")
        nc.sync.dma_start(bk_t[:], bkd.ap())
        bv_t = []
        for i in range(2):
            t = sbc.tile([128, 1], F32, tag=f"bv{i}", name=f"bv{i}")
            nc.sync.dma_start(t[:], bvd.ap()[i * 128:(i + 1) * 128, :])
            bv_t.append(t)

        # weights -> SBUF, rounded to f32r
        wq_r, wk_r, wv_r = [], [], []
        for k in range(2):
            for name, dram, width, lst in (
                ("wq", wqT, DQK, wq_r), ("wk", wkT, DQK, wk_r), ("wv", wvT, C, wv_r),
            ):
                raw = sbx.tile([128, width], F32, tag="wraw")
                nc.sync.dma_start(raw[:], dram.ap()[k * 128:(k + 1) * 128, :])
                t = sbc.tile([128, width], F32R, tag=f"{name}{k}", name=f"{name}{k}")
                nc.vector.tensor_copy(t[:], raw[:])
                lst.append(t)

        # activations -> SBUF, rounded to f32r
        x2_r, x1_r = [], []
        for k in range(2):
            raw = sbx.tile([128, N], F32, tag="xraw")
            nc.sync.dma_start(raw[:], x2s.ap()[k * 128:(k + 1) * 128, :])
            t = sbx.tile([128, N], F32R, tag="x2r", name=f"x2r{k}")
            nc.vector.tensor_copy(t[:], raw[:])
            x2_r.append(t)
        for k in range(2):
            raw = sbx.tile([128, MH], F32, tag="xraw")
            nc.sync.dma_start(raw[:], x1s.ap()[k * 128:(k + 1) * 128, :])
            t = sbx.tile([128, MH], F32R, tag="x1r", name=f"x1r{k}")
            nc.vector.tensor_copy(t[:], raw[:])
            x1_r.append(t)

        # K' = fold(Wk) @ x2 + bk   -> [32, N] f32r
        ksb = sbc.tile([DQK, N], F32R, tag="ksb")
        for cch in range(N // 512):
            pk = psa.tile([DQK, 512], F32, tag="st")
            for k in range(2):
                nc.tensor.matmul(pk[:], wk_r[k][:], x2_r[k][:, cch * 512:(cch + 1) * 512],
                                 start=(k == 0), stop=(k == 1))
            nc.vector.tensor_scalar_add(ksb[:, cch * 512:(cch + 1) * 512], pk[:], bk_t[:])

        # Q' = fold(Wq) @ x1_half + bq  (scale folded) -> [32, MH] f32r
        qsb = sbc.tile([DQK, MH], F32R, tag="qsb")
        for cch in range(MH // 512):
            pq = psa.tile([DQK, 512], F32, tag="st")
            for k in range(2):
                nc.tensor.matmul(pq[:], wq_r[k][:], x1_r[k][:, cch * 512:(cch + 1) * 512],
                                 start=(k == 0), stop=(k == 1))
            nc.vector.tensor_scalar_add(qsb[:, cch * 512:(cch + 1) * 512], pq[:], bq_t[:])

        # V^T tiles: vt[:, nt, c] = (x2^T Wv^T)[n, c]  (no bias) -> [128, NT, C] f32r
        vt = sbc.tile([128, NT, C], F32R, tag="vt")
        for nt in range(NT):
            pv = psa.tile([128, C], F32, tag="st")
            for k in range(2):
                nc.tensor.matmul(pv[:], x2_r[k][:, nt * 128:(nt + 1) * 128], wv_r[k][:],
                                 start=(k == 0), stop=(k == 1))
            nc.vector.tensor_copy(vt[:, nt, :], pv[:])

        # main loop: S^T tiles -> exp -> PV accumulate (+rowsum)
        for mq in range(MQ):
            msl = slice(mq * MQS, (mq + 1) * MQS)
            oc = [pso.tile([128, MQS], F32, tag="outc", name=f"oc{mq}_{i}") for i in range(2)]
            rs = psr.tile([1, MQS], F32, tag="rowsum")
            for nt in range(NT):
                st = psa.tile([128, MQS], F32, tag="st")
                nc.tensor.matmul(st[:], ksb[:, nt * 128:(nt + 1) * 128], qsb[:, msl],
                                 start=True, stop=True)
                pt = sbp.tile([128, MQS], F32R, tag="pt")
                nc.scalar.activation(pt[:], st[:], Exp)
                first, last = nt == 0, nt == NT - 1
                for ci in range(2):
                    nc.tensor.matmul(oc[ci][:], vt[:, nt, ci * 128:(ci + 1) * 128], pt[:],
                                     start=first, stop=last)
                nc.tensor.matmul(rs[:], ones_r[:], pt[:], start=first, stop=last)
            # finalize: out = oc / rowsum + bv
            recip = sbp.tile([1, MQS], F32, tag="recip")
            nc.vector.reciprocal(recip[:], rs[:])
            recipb = sbp.tile([128, MQS], F32, tag="recipb")
            nc.gpsimd.partition_broadcast(recipb[:], recip[:])
            for ci in range(2):
                yt = sbp.tile([128, MQS], F32, tag="y")
                nc.vector.tensor_mul(yt[:], oc[ci][:], recipb[:])
                nc.vector.tensor_scalar_add(yt[:], yt[:], bv_t[ci][:])
                nc.sync.dma_start(y.ap()[ci * 128:(ci + 1) * 128, msl], yt[:])

    nc.compile()
    return nc


def _fold_weights(w, b, gamma, beta, mean, var, scale=1.0):
    w = w.astype(np.float64)
    inv = gamma.astype(np.float64) / np.sqrt(var.astype(np.float64) + EPS)
    shift = beta.astype(np.float64) - mean.astype(np.float64) * inv
    wf = w * inv[:, None] * scale
    bf = (b.astype(np.float64) * inv + shift) * scale
    return (np.ascontiguousarray(wf.T).astype(np.float32),
            bf.astype(np.float32)[:, None])


def kernel(x1, x2, q_w, q_b, q_gamma, q_beta, q_mean, q_var,
           k_w, k_b, k_gamma, k_beta, k_mean, k_var,
           v_w, v_b, v_gamma, v_beta, v_mean, v_var):
    global _prog, LAST_RESULTS
    if _prog is None:
        _prog = _build()

    s = 1.0 / np.sqrt(np.float64(DQK))
    WqT, bq = _fold_weights(q_w, q_b, q_gamma, q_beta, q_mean, q_var, s)
    WkT, bk = _fold_weights(k_w, k_b, k_gamma, k_beta, k_mean, k_var)
    WvT, bv = _fold_weights(v_w, v_b, v_gamma, v_beta, v_mean, v_var)

    x1f = np.asarray(x1, dtype=np.float32).reshape(B, C, N)
    x2f = np.asarray(x2, dtype=np.float32).reshape(B, C, N)

    in_maps = []
    for core in range(NCORES):
        b, h = divmod(core, 2)
        in_maps.append({
            "x1s": np.ascontiguousarray(x1f[b][:, h * MH:(h + 1) * MH]),
            "x2s": np.ascontiguousarray(x2f[b]),
            "wqT": WqT, "wkT": WkT, "wvT": WvT,
            "bq": bq, "bk": bk, "bv": bv,
        })

    LAST_RESULTS = run_bass_kernel_spmd(_prog, in_maps, core_ids=list(range(NCORES)))
    out = np.empty((B, C, N), np.float32)
    for core in range(NCORES):
        b, h = divmod(core, 2)
        out[b][:, h * MH:(h + 1) * MH] = LAST_RESULTS.results[core]["y"]
    return out.reshape(B, C, Hs, Ws)
